# revision 15
# baseline (speedup 1.0000x reference)
# GCN layer kernel for Trainium2: out[b] = relu((a[b] @ x[b]) @ W) * mask[b]
#
# Sharding: data-parallel over the batch (graph) dim. B=8 graphs, 8 cores,
# one graph per core; W replicated. Inputs are the FULL tensors; shards are
# sliced host-side and the per-core outputs stacked back together.
#
# Per-core dataflow (a: [2048,2048], x: [2048,512], W: [512,512]):
#   - All matmuls in bf16 (fp32 PSUM accumulation); rel-err ~3e-3, tol 2e-2.
#   - a is used transposed (contraction over its column index m): strips
#     cast to bf16 on DVE (eagerly, in arrival order - lazy casts throttle
#     the load ring through the landing-pool WAR), PE-transposed in quads
#     of [128,128] tiles through PSUM, copyback alternating DVE/ACT into
#     at[p, mi, ni, j] = a[n0+j, 128mi+p].
#   - DMA plan (measured): ONE sync-ring load stream in priority order
#     x0, x1, c0 strips, x2..x15, c1, c2, c3; W on gpsimd as a cast-DMA;
#     scalar ring carries only output stores. Strips (8KB descriptors)
#     move at ~390 GB/s, x row-tiles (2KB descriptors) at ~210; ~9us of
#     engine boot precedes any DMA. Measured dead ends: x via gpsimd
#     SWDGE runs at only ~50-90 GB/s; splitting loads across both HWDGE
#     rings drops aggregate to ~270 GB/s (2KB packets poison the SDMA
#     round-robin); DMA-xbar transposes shatter into 320B packets.
#   - mm1 is mi-outer everywhere (4 PSUM banks accumulate tT[fi] across
#     mi): chunk 0 starts as soon as its strips are transposed (~18us)
#     and consumes x tiles as they trickle in. Chunk nj's mm2 waves and
#     chunk nj+1's transpose quads run in the boundary after mm1(nj) -
#     the DMA cadence (chunk nj+1 lands ~11.7us after nj) leaves no
#     slack to ride them inside the mm1 stream itself. Chunk 3's mm2 is
#     a single fi-major merged wave across 4 banks (ps_o + ps_tp) to
#     shrink the tail.
#   - HAM: the PE clock-gates to half speed after any ~3.4us-idle window
#     (transposes do NOT count as activity): 20 identity warms cover the
#     boot window, warm matmuls chained to strip/x casts cover the rest
#     of the DMA window.

import numpy as np

B, N, F, D = 8, 2048, 512, 512
P = 128
NT = N // P        # 16 row-tiles of n (and of m, since a is square)
FT = F // P        # 4 tiles of f
NCHUNK = 512       # n is processed in chunks of 512 rows
NJ = N // NCHUNK   # 4
NSUB = NCHUNK // P # 4

_CACHE = {}


def _build_nc():
    from contextlib import ExitStack

    from concourse import bacc, mybir, tile
    from concourse.masks import make_identity

    f32 = mybir.dt.float32
    bf16 = mybir.dt.bfloat16
    AF = mybir.ActivationFunctionType

    nc = bacc.Bacc(None)
    a_d = nc.dram_tensor("a", [N, N], f32, kind="ExternalInput")
    x_d = nc.dram_tensor("x", [N, F], f32, kind="ExternalInput")
    w_d = nc.dram_tensor("kernel", [F, D], f32, kind="ExternalInput")
    o_d = nc.dram_tensor("out", [N, D], f32, kind="ExternalOutput")

    with tile.TileContext(nc) as tc, ExitStack() as ctx:
        const = ctx.enter_context(tc.tile_pool(name="const", bufs=1))
        xp = ctx.enter_context(tc.tile_pool(name="xp", bufs=1))
        wp = ctx.enter_context(tc.tile_pool(name="wp", bufs=1))
        xlp = ctx.enter_context(tc.tile_pool(name="xlp", bufs=1))
        drp = ctx.enter_context(tc.tile_pool(name="drp", bufs=1, space="DRAM"))
        afp = ctx.enter_context(tc.tile_pool(name="afp", bufs=6))
        abp = ctx.enter_context(tc.tile_pool(name="abp", bufs=6))
        atp = ctx.enter_context(tc.tile_pool(name="atp", bufs=2))
        ttp = ctx.enter_context(tc.tile_pool(name="ttp", bufs=2))
        outp = ctx.enter_context(tc.tile_pool(name="outp", bufs=3))
        scr = ctx.enter_context(tc.tile_pool(name="scr", bufs=2))
        ps_mm = ctx.enter_context(tc.tile_pool(name="ps_mm", bufs=4, space="PSUM"))
        ps_o = ctx.enter_context(tc.tile_pool(name="ps_o", bufs=2, space="PSUM"))
        ps_tp = ctx.enter_context(tc.tile_pool(name="ps_tp", bufs=2, space="PSUM"))

        ident = const.tile([P, P], f32)
        make_identity(nc, ident[:])
        ident_b = const.tile([P, P], bf16)
        nc.vector.tensor_copy(ident_b[:], ident[:])

        x_b = xp.tile([P, NT, F], bf16)
        w_b = wp.tile([P, FT, D], bf16)
        sumabs = const.tile([P, NT], f32)
        mask_sb = const.tile([P, NT], f32)

        def warm_fp32():
            pw = ps_o.tile([P, D], f32, tag="pso", name="pw")
            nc.tensor.matmul(
                pw[:, :P], lhsT=ident[:], rhs=ident[:], start=True, stop=True
            )

        def warm_bf16(lhs, rhs):
            # fires as the just-cast tile lands; paces PE activity (HAM)
            # through the DMA window.
            pw = ps_o.tile([P, D], f32, tag="pso", name="pwb")
            nc.tensor.matmul(
                pw[:, : rhs.shape[-1]], lhsT=lhs, rhs=rhs, start=True, stop=True
            )

        at_tiles = [None] * NJ

        def at_of(nj):
            if at_tiles[nj] is None:
                at_tiles[nj] = atp.tile(
                    [P, NT, NSUB, P], bf16, tag="at", name=f"at{nj}"
                )
            return at_tiles[nj]

        # ---------- loads ----------
        # x loads in the INTERLEAVED layout x_b[p, o, f] = x[16p+o, f]:
        # each partition gets 4 consecutive DRAM rows per granule -> 8KB
        # contiguous descriptors at strip speed (~390 GB/s) instead of
        # the 2KB row-tile descriptors (~210). mm1 contracts over m in
        # the SAME permuted order on both operands (the a-transposes use
        # stride-16 column sets), so the result is unchanged.
        x_if = xlp.tile([P, NT, F], f32)
        x_iv = x_d[:].rearrange("(p o) f -> p o f", o=16)

        def load_x_gran(k):
            nc.sync.dma_start(
                x_if[:, 4 * k : 4 * k + 4, :], x_iv[:, 4 * k : 4 * k + 4, :]
            )

        def load_strip(nj, ni):
            af = afp.tile([P, N], f32, tag="af", name="af")
            r0 = (nj * NSUB + ni) * P
            nc.sync.dma_start(af[:], a_d[r0 : r0 + P, :])
            return af

        for _ in range(20):
            warm_fp32()

        load_x_gran(0)
        af0 = [load_strip(0, ni) for ni in range(NSUB)]
        for k in range(1, 4):
            load_x_gran(k)
        af1 = [load_strip(1, ni) for ni in range(NSUB)]
        af2 = [load_strip(2, ni) for ni in range(NSUB)]
        af3 = [load_strip(3, ni) for ni in range(NSUB)]
        afs = [af0, af1, af2, af3]
        nc.gpsimd.dma_start(w_b[:], w_d[:].rearrange("(o p) d -> p o d", p=P))

        # ---------- preamble: chunk 0 cast + PE transpose ----------
        def cast_x_gran(k):
            nc.vector.tensor_copy(
                x_b[:, 4 * k : 4 * k + 4, :], x_if[:, 4 * k : 4 * k + 4, :]
            )

        cast_x_gran(0)
        warm_bf16(x_b[:, 0, 0:P], x_b[:, 0, :])
        warm_bf16(x_b[:, 1, 0:P], x_b[:, 1, :])

        abs_ = {}  # (nj, ni) -> bf16 strip

        def cast_strip(nj, ni, warm=False):
            ab = abp.tile([P, N], bf16, tag="ab", name=f"ab{nj}_{ni}")
            nc.vector.tensor_copy(ab[:], afs[nj][ni][:])
            if warm:
                warm_bf16(ab[:, 0:P], ab[:, 0:NCHUNK])
                warm_bf16(ab[:, P : 2 * P], ab[:, NCHUNK : 2 * NCHUNK])
            abs_[(nj, ni)] = ab

        cbn = 0  # copyback DVE/ACT alternation

        def t_quad(nj, slot):
            # PE-transpose 4 tiles (strip ni, m-tiles q*4..q*4+3) through one
            # PSUM bank, then DVE/ACT copy into at[p, mtile, ni, r].
            nonlocal cbn
            ni, q = divmod(slot, 4)
            abr = abs_[(nj, ni)][:].rearrange("p (c o) -> p o c", o=16)
            ps = ps_tp.tile([P, NCHUNK], bf16, tag="pst", name="pst")
            for k in range(4):
                o = q * 4 + k
                nc.tensor.transpose(
                    ps[:, k * P : (k + 1) * P], abr[:, o, :], ident_b[:]
                )
            src = ps[:].rearrange("p (a f) -> p a f", a=4)
            dst = at_of(nj)[:, q * 4 : (q + 1) * 4, ni, :]
            if cbn % 2 == 0:
                nc.vector.tensor_copy(dst, src)
            else:
                nc.scalar.copy(dst, src)
            cbn += 1

        def abs_gran(k):
            for o in range(4 * k, 4 * k + 4):
                abs_scr = scr.tile([P, F], bf16, tag="abs_scr")
                nc.scalar.activation(
                    abs_scr[:], x_b[:, o, :], AF.Abs,
                    accum_out=sumabs[:, o : o + 1],
                )

        for ni in range(NSUB):
            cast_strip(0, ni, warm=True)
            if ni >= 1:
                cast_x_gran(ni)
            for q in range(4):
                t_quad(0, 4 * ni + q)

        # mask in interleaved layout, fixed up via a DRAM roundtrip:
        # sumabs[p, o] belongs to row 16p+o; relu needs mask[p, nt] for
        # row 128nt+p. Store row-major, reload with the other split.
        for k in range(4):
            abs_gran(k)
        mask_i = const.tile([P, NT], f32)
        nc.vector.tensor_scalar(
            mask_i[:], sumabs[:], 0.0, None, mybir.AluOpType.is_gt
        )
        mscr = drp.tile([N], f32)
        nc.scalar.dma_start(mscr[:].rearrange("(p o) -> p o", o=16), mask_i[:])
        nc.scalar.dma_start(mask_sb[:], mscr[:].rearrange("(nt p) -> p nt", p=P))

        # ---------- main loop ----------
        tts = [None] * NJ

        def tt_copy(nj, pt, fi):
            if tts[nj] is None:
                tts[nj] = ttp.tile([P, FT, NCHUNK], bf16, tag="tt", name=f"tt{nj}")
            nc.scalar.copy(tts[nj][:, fi], pt[fi][:])

        def mm1_chunk(nj):
            at = at_of(nj)
            pt = [
                ps_mm.tile([P, NCHUNK], f32, tag="psm", name=f"pt_{nj}_{fi}")
                for fi in range(FT)
            ]
            for mi in range(NT):
                for fi in range(FT):
                    nc.tensor.matmul(
                        pt[fi][:],
                        lhsT=x_b[:, mi, fi * P : (fi + 1) * P],
                        rhs=at[:, mi, :, :],
                        start=(mi == 0),
                        stop=(mi == NT - 1),
                    )
            for fi in range(FT):
                tt_copy(nj, pt, fi)

        def mm2_wave(nj, w):
            # half of mm2 for chunk nj: output tiles ns = 2w, 2w+1,
            # accumulated over fi in 2 ps_o banks, then fused
            # relu(mask * po) -> SBUF -> store (2 row-tiles per DMA).
            tt = tts[nj]
            pos = [
                ps_o.tile([P, D], f32, tag="pso", name=f"po_{nj}_{w}_{i}")
                for i in range(2)
            ]
            for fi in range(FT):
                for i in range(2):
                    ns = 2 * w + i
                    nc.tensor.matmul(
                        pos[i][:],
                        lhsT=tt[:, fi, ns * P : (ns + 1) * P],
                        rhs=w_b[:, fi],
                        start=(fi == 0),
                        stop=(fi == FT - 1),
                    )
            store_pair(nj, w, pos)

        def store_pair(nj, w, pos, q=None):
            ob = outp.tile([P, 2, D], f32, tag="ob")
            for i in range(2):
                ni = nj * NSUB + 2 * w + i
                nc.scalar.activation(
                    ob[:, i, :], pos[i][:], AF.Relu, scale=mask_sb[:, ni : ni + 1]
                )
            r0 = (nj * NSUB + 2 * w) * P
            dst = o_d[r0 : r0 + 2 * P, :].rearrange("(t p) d -> p t d", p=P)
            (q or nc.scalar).dma_start(dst, ob[:])

        # chunk 1's first strips cast eagerly (they land during mm1 c0)
        cast_strip(1, 0)
        cast_strip(1, 1)
        cast_strip(1, 2)

        mm1_chunk(0)

        warm_bf16(x_b[:, 0, 0:P], x_b[:, 0, :])
        warm_bf16(x_b[:, 1, 0:P], x_b[:, 1, :])

        # boundaries: mm2 waves of the finished chunk + quads of the next
        for nj in range(NJ - 1):
            nxt = nj + 1
            mm2_wave(nj, 0)
            for slot in range(8):
                t_quad(nxt, slot)
            cast_strip(nxt, 3)
            mm2_wave(nj, 1)
            for slot in range(8, 16):
                t_quad(nxt, slot)
            if nxt < NJ - 1:
                cast_strip(nxt + 1, 0)
                cast_strip(nxt + 1, 1)
                cast_strip(nxt + 1, 2)
            mm1_chunk(nxt)

        # --- chunk 3 mm2: merged fi-major wave across 4 banks for a
        #     minimal tail (quads done -> ps_tp is free) ---
        pos3 = [
            (ps_o if i < 2 else ps_tp).tile(
                [P, D], f32, tag=("pso" if i < 2 else "pst"), name=f"po3_{i}"
            )
            for i in range(4)
        ]
        tt3 = tts[3]
        for fi in range(FT):
            for i in range(4):
                nc.tensor.matmul(
                    pos3[i][:],
                    lhsT=tt3[:, fi, i * P : (i + 1) * P],
                    rhs=w_b[:, fi],
                    start=(fi == 0),
                    stop=(fi == FT - 1),
                )
        store_pair(3, 0, pos3[0:2], q=nc.scalar)
        store_pair(3, 1, pos3[2:4], q=nc.sync)

    nc.compile()
    return nc


def get_nc():
    if "nc" not in _CACHE:
        _CACHE["nc"] = _build_nc()
    return _CACHE["nc"]


def kernel(**inputs) -> np.ndarray:
    from concourse.bass_utils import run_bass_kernel_spmd

    x = np.ascontiguousarray(np.asarray(inputs["x"], dtype=np.float32))
    a = np.ascontiguousarray(np.asarray(inputs["a"], dtype=np.float32))
    w = np.ascontiguousarray(np.asarray(inputs["kernel"], dtype=np.float32))
    assert x.shape == (B, N, F) and a.shape == (B, N, N) and w.shape == (F, D)

    nc = get_nc()
    in_maps = [{"a": a[b], "x": x[b], "kernel": w} for b in range(B)]
    res = run_bass_kernel_spmd(nc, in_maps, core_ids=list(range(B)))
    return np.stack([res.results[b]["out"] for b in range(B)], axis=0)
